# revision 1
# baseline (speedup 1.0000x reference)
"""Trainium2 Bass kernel for a 2-layer dense GCN (NodeEncoder).

    out = adj @ relu(adj @ (x@W1) + b1) @ W2 + b2
    N=16384, F_IN=512, HID=1024, OUT=256, adj dense [N, N] fp32.

Sharding: adj row-partitioned across 8 NeuronCores (2048 rows/core).
All device matmuls consume natural-layout (row-major) operands; the
host pre-transposes adj/x per shard so no on-device transposes are
needed.  Per core:

  phase A:  s1_c   = x_c @ W1                  [2048, 1024]  (own rows)
  AG1:      s1     = AllGather(s1_c)           [16384, 1024]
  phase B:  hT_c   = relu(adj_c @ s1 + b1)^T   [1024, 2048]  (transposed
            orientation: lhsT = s1 tiles, rhs = adjT_c tiles -> psum is
            [n, m]; bias b1 is per-partition, fused into the ACT relu)
  phase C:  s2_c   = h_c @ W2                  [2048, 256]   (lhsT = hT_c)
  AG2:      s2     = AllGather(s2_c)           [16384, 256]
  phase D:  out2T_c = (adj_c @ s2)^T + b2      [256, 2048]   (lhsT = s2
            tiles, rhs = adjT_c tiles; b2 per-partition via ACT Copy)

Matmuls run in bf16 with fp32 PSUM accumulation (max rel err vs fp32
reference ~3e-3 of absmax).
"""

import numpy as np
import ml_dtypes

import concourse.bass as bass
import concourse.mybir as mybir
import concourse.tile as tile
from concourse.bass_utils import run_bass_kernel_spmd
from concourse.tile_sem_assignment import N_PROCS
from concourse.vector_clock import ScopedClock, VectorClock
from concourse.tile_rust import add_dep_helper as tile_rust_add_dep

# ---------------------------------------------------------------------------
# Workaround: the walrus build in this container caps the number of sync-wait
# commands on a Drain instruction; Tile's kernel-tail drain aggregates one
# wait per logical processor and exceeds it.  Split the tail drain into a
# chain of single-wait drains on the same (SP) queue — semantically identical.
# ---------------------------------------------------------------------------


def _drain_and_barrier_split(self, tick_clock, wait_clock):
    gc = tick_clock.global_clock
    for p in range(N_PROCS):
        partial = VectorClock([gc[q] if q == p else 0 for q in range(N_PROCS)])
        d = self.nc.sync.drain()
        wait_clock.add_sem_waits(d.ins, ScopedClock({None: partial}))
    self.nc.sync.drain()

    self.nc.all_engine_barrier()
    assert self.sems is not None
    popped = self.nc._tile_sem_poison_stack.pop()
    assert popped is self._sem_poison
    self.nc.clear_and_free_semaphores(list(self.sems.allocated().values()))
    self.nc.all_engine_barrier()


tile.TileContext._drain_and_barrier = _drain_and_barrier_split

# The same walrus cap applies to every instruction kind: at most ONE sync
# wait command per instruction (probed empirically — a 2-wait TensorCopy is
# rejected).  Post-pass: hoist excess sem-waits onto no-ops inserted just
# before the instruction on the same engine queue — per-engine program order
# makes this semantically identical.
_MAX_WAITS = 1


def _split_excess_waits(nc):
    ctr = 0
    for f in nc.m.functions:
        for bb in f.blocks:
            out = []
            changed = False
            for inst in bb.instructions:
                si = inst.sync_info
                waits = list(si.on_wait) if si is not None and si.on_wait else []
                if len(waits) > _MAX_WAITS:
                    changed = True
                    keep, excess = waits[: _MAX_WAITS], waits[_MAX_WAITS :]
                    for i in range(0, len(excess), _MAX_WAITS):
                        ctr += 1
                        nop = mybir.InstNoOp(name=f"I-waitnop-{ctr}")
                        nop.engine = inst.engine
                        nop.sync_info = mybir.SyncInfo(
                            on_wait=excess[i : i + _MAX_WAITS], on_update=[]
                        )
                        out.append(nop)
                    si.on_wait = keep
                out.append(inst)
            if changed:
                bb.instructions = out
    return ctr

def _elide_redundant_ldweights(nc):
    """Delete an InstLdweights that reloads the exact weights AP loaded by
    the previous (surviving) InstLdweights when only plain matmuls / no-ops
    sit between them in the scheduled stream.  The PE array keeps the
    stationary operand across matmuls, so the reload is pure overhead
    (walrus emits one LDWEIGHTS per MATMUL and its ldw-opt pass is
    incompatible with pre-split LDW+MM).  Only sync-free LDWs are removed,
    so semaphore bookkeeping is unchanged."""
    n_elided = 0
    for f in nc.m.functions:
        for bb in f.blocks:
            out = []
            last_w = None  # weights-AP repr of last surviving LDW, if run intact
            changed = False
            for inst in bb.instructions:
                nm = type(inst).__name__
                if nm == "InstLdweights":
                    si = inst.sync_info
                    clean = not (si and (si.on_wait or si.on_update))
                    w = repr(inst.ins[0])
                    if clean and last_w == w:
                        n_elided += 1
                        changed = True
                        continue  # drop the reload
                    last_w = w if clean else None
                elif nm == "InstMatmult":
                    if getattr(inst, "is_transpose", False):
                        last_w = None
                elif nm == "InstNoOp":
                    pass
                else:
                    last_w = None
                out.append(inst)
            if changed:
                bb.instructions = out
    return n_elided


NCORES = 8
N = 16384
SH = N // NCORES  # 2048 adj rows per core
F = 512
HID = 1024
OUT = 256

BF16 = mybir.dt.bfloat16
F32 = mybir.dt.float32

_built = None


def build():
    """Build the per-core Bass program (identical on all cores)."""
    nc = bass.Bass()

    adjT = nc.declare_dram_parameter("adjT", [N, SH], BF16, isOutput=False)
    xT = nc.declare_dram_parameter("xT", [F, SH], BF16, isOutput=False)
    w1 = nc.declare_dram_parameter("w1", [F, HID], BF16, isOutput=False)
    w2 = nc.declare_dram_parameter("w2", [HID, OUT], BF16, isOutput=False)
    b1T = nc.declare_dram_parameter("b1T", [128, HID // 128], F32, isOutput=False)
    b2T = nc.declare_dram_parameter("b2T", [128, OUT // 128], F32, isOutput=False)
    out2T = nc.declare_dram_parameter("out2T", [OUT, SH], F32, isOutput=True)

    rg = [list(range(NCORES))]

    # adjT column-block mb (512 wide), 4 k-blocks per DMA:
    #   [p, k4, kk, m] = adjT[k4*512 + kk*128 + p, mb*512 + m]
    def adjT_src(mb):
        return adjT[:, mb * 512 : (mb + 1) * 512].rearrange(
            "(k4 kk p) m -> p k4 kk m", kk=4, p=128
        )

    def adjTp_src(mbp):
        return adjT[:, mbp * 1024 : (mbp + 1) * 1024].rearrange(
            "(k4 kk p) m -> p k4 kk m", kk=4, p=128
        )

    def allgather(inp, outp):
        return nc.gpsimd.collective_compute(
            "AllGather",
            mybir.AluOpType.bypass,
            replica_groups=rg,
            ins=[inp.opt()],
            outs=[outp.opt()],
        )

    with tile.TileContext(nc) as tc:
        with (
            tc.tile_pool(name="const", bufs=1) as constp,
            tc.tile_pool(name="psum", bufs=8, space="PSUM") as psum,
            tc.tile_pool(name="dram", bufs=1, space="DRAM") as dram,
            tc.tile_pool(name="adj", bufs=4) as adjp,
            tc.tile_pool(name="small", bufs=4) as smallp,
        ):
            # ---- constants ----
            w2t = constp.tile([128, HID // 128, OUT], BF16)
            nc.sync.dma_start(w2t[:], w2[:].rearrange("(f p) n -> p f n", p=128))
            b1t = constp.tile([128, HID // 128], F32)
            nc.sync.dma_start(b1t[:], b1T[:])
            b2t = constp.tile([128, OUT // 128], F32)
            nc.sync.dma_start(b2t[:], b2T[:])

            # AllGathers split in quarters so they overlap compute: phase B
            # can start once the first two s1 quarters have gathered, and
            # phase D streams k-blocks in gather-arrival order.
            ag1_in = [dram.tile([SH, 512], BF16, name=f"ag1i{h}") for h in range(2)]
            ag1_out = [
                dram.tile([N, 512], BF16, addr_space="Shared", name=f"ag1o{h}")
                for h in range(2)
            ]
            ag2_in = [dram.tile([SH // 4, OUT], BF16, name=f"ag2i{q}") for q in range(4)]
            ag2_out = [
                dram.tile([N // 4, OUT], BF16, addr_space="Shared", name=f"ag2o{q}")
                for q in range(4)
            ]

            # ---- phase A: s1_c = x_c @ W1 (per n-quarter; AG per quarter) ----
            with tc.tile_pool(name="phA", bufs=1) as pA:
                xt = []
                w1t = []
                for f in range(4):
                    t = pA.tile([128, SH], BF16, name=f"xt{f}")
                    nc.sync.dma_start(t[:], xT[f * 128 : (f + 1) * 128, :])
                    xt.append(t)
                    t = pA.tile([128, HID], BF16, name=f"w1t{f}")
                    nc.sync.dma_start(t[:], w1[f * 128 : (f + 1) * 128, :])
                    w1t.append(t)
                # half 0 gathers immediately (it gates phase B's start);
                # half 1 is computed now but gathered later (delayed dep)
                for h in range(2):
                    for mt in range(SH // 128):
                        psa = psum.tile([128, 512], F32, tag="ps", name=f"psA{h}{mt}")
                        for f in range(4):
                            nc.tensor.matmul(
                                psa[:],
                                xt[f][:, mt * 128 : (mt + 1) * 128],
                                w1t[f][:, h * 512 : (h + 1) * 512],
                                start=(f == 0),
                                stop=(f == 3),
                            )
                        s1o = smallp.tile([128, 512], BF16, tag="s1o", bufs=2)
                        nc.vector.tensor_copy(s1o[:], psa[:])
                        nc.scalar.dma_start(
                            ag1_in[h][mt * 128 : (mt + 1) * 128, :], s1o[:]
                        )
                    if h == 0:
                        allgather(ag1_in[0], ag1_out[0])

            # ---- phases B + C (C quarters interleaved so AG2 fires early) --
            with (
                tc.tile_pool(name="s1res", bufs=32) as s1p,
                tc.tile_pool(name="ht", bufs=32) as htp,
            ):
                ht_tiles = {}

                def phase_c_quarter(qq):
                    # s2 rows qq*512 .. +511 (needs ht tiles mb=qq, all f)
                    for mth in range(4):
                        mt = qq * 4 + mth
                        mb, off = mt // 4, (mt % 4) * 128
                        psc = psum.tile([128, OUT], F32, tag="ps", name=f"psC{mt}")
                        for f in range(8):
                            nc.tensor.matmul(
                                psc[:],
                                ht_tiles[(f, mb)][:, off : off + 128],
                                w2t[:, f, :],
                                start=(f == 0),
                                stop=(f == 7),
                            )
                        s2o = smallp.tile([128, OUT], BF16, tag="s2o", bufs=2)
                        nc.vector.tensor_copy(s2o[:], psc[:])
                        nc.scalar.dma_start(
                            ag2_in[qq][mth * 128 : (mth + 1) * 128, :], s2o[:]
                        )
                    allgather(ag2_in[qq], ag2_out[qq])

                for nh in range(2):
                    s1_src = ag1_out[nh][:].rearrange(
                        "(k4 kk p) n -> p k4 kk n", kk=4, p=128
                    )
                    s1t = []
                    # m-blocks processed in pairs: each stationary s1 slice
                    # feeds 2 matmuls (adjacent mb), halving LDWEIGHTS count.
                    for mbp in range(2):
                        ps = [
                            psum.tile(
                                [128, 512], F32, tag="ps", name=f"psB{nh}{mbp}{i}"
                            )
                            for i in range(8)
                        ]  # index nt*2 + mbx
                        for k4 in range(32):
                            if mbp == 0:
                                t = s1p.tile(
                                    [128, 4, 512], BF16, tag="s1t",
                                    name=f"s1t{nh}{k4}",
                                )
                                nc.sync.dma_start(t[:], s1_src[:, k4])
                                s1t.append(t)
                            ats = []
                            for mbx in range(2):
                                atx = adjp.tile(
                                    [128, 4, 512], BF16, tag="adjt", bufs=4,
                                    name=f"at{nh}{mbp}{k4}{mbx}",
                                )
                                nc.sync.dma_start(
                                    atx[:], adjT_src(mbp * 2 + mbx)[:, k4]
                                )
                                ats.append(atx)
                            for kk in range(4):
                                k = k4 * 4 + kk
                                for nt in range(4):
                                    lhs = s1t[k4][:, kk, nt * 128 : (nt + 1) * 128]
                                    for mbx in range(2):
                                        nc.tensor.matmul(
                                            ps[nt * 2 + mbx][:],
                                            lhs,
                                            ats[mbx][:, kk, :],
                                            start=(k == 0),
                                            stop=(k == 127),
                                        )
                        last_act = None
                        for nt in range(4):
                            j = nh * 4 + nt
                            for mbx in range(2):
                                mb = mbp * 2 + mbx
                                htt = htp.tile([128, 512], BF16, tag="htt")
                                last_act = nc.scalar.activation(
                                    htt[:],
                                    ps[nt * 2 + mbx][:],
                                    mybir.ActivationFunctionType.Relu,
                                    bias=b1t[:, j : j + 1],
                                )
                                ht_tiles[(j, mb)] = htt
                        if nh == 0 and mbp == 0:
                            # fire the second-half s1 gather now; dep delays
                            # its SDMA traffic past B's startup loads
                            cc = allgather(ag1_in[1], ag1_out[1])
                            tile_rust_add_dep(
                                cc.ins,
                                last_act.ins,
                                sync=True,
                                reason="delay s1 half-1 gather past B start",
                            )
                        if nh == 1:
                            # ht tiles for mb 2*mbp..2*mbp+1 now complete for
                            # all f -> emit the matching C quarters + gathers.
                            phase_c_quarter(2 * mbp)
                            phase_c_quarter(2 * mbp + 1)

            # ---- phase D: out2T = (adj_c @ s2)^T + b2 ----
            # All 8 psum banks accumulate concurrently; k-blocks consumed in
            # gather-arrival order (quarter-major), s2 tiles loaded JIT after
            # each adjT chunk so the SP queue stays load-ordered.
            with (
                tc.tile_pool(name="s2res", bufs=32) as s2p,
                tc.tile_pool(name="adjD", bufs=4) as adjDp,
                tc.tile_pool(name="outp", bufs=8) as outp,
            ):
                # ag2_out[qq] rows = g*512 + skk*128 + p  (rank g, block qq)
                s2_srcs = [
                    ag2_out[qq][:].rearrange("(g skk p) n -> p g skk n", g=8, p=128)
                    for qq in range(4)
                ]
                adjD_src = adjT[:].rearrange("(k4 kk p) m -> p k4 kk m", kk=4, p=128)
                dps = [
                    psum.tile([128, 512], F32, tag="ps", name=f"psD{i}")
                    for i in range(8)
                ]
                # k4 = g*4 + qq  ->  iterate quarter-major
                k4_order = [g * 4 + qq for qq in range(4) for g in range(8)]
                for ki, k4 in enumerate(k4_order):
                    g, qq = k4 // 4, k4 % 4
                    at = adjDp.tile([128, 4, SH], BF16, tag="adjD", name=f"atD{k4}")
                    nc.sync.dma_start(at[:], adjD_src[:, k4])
                    st = s2p.tile([128, 4, OUT], BF16, tag="s2t", name=f"s2t{k4}")
                    nc.sync.dma_start(st[:], s2_srcs[qq][:, g])
                    for kk in range(4):
                        for n2t in range(2):
                            lhs = st[:, kk, n2t * 128 : (n2t + 1) * 128]
                            for mb in range(4):
                                nc.tensor.matmul(
                                    dps[n2t * 4 + mb][:],
                                    lhs,
                                    at[:, kk, mb * 512 : (mb + 1) * 512],
                                    start=(ki == 0 and kk == 0),
                                    stop=(ki == 31 and kk == 3),
                                )
                for n2t in range(2):
                    for mb in range(4):
                        ot = outp.tile([128, 512], F32, tag="ot")
                        nc.scalar.activation(
                            ot[:],
                            dps[n2t * 4 + mb][:],
                            mybir.ActivationFunctionType.Identity,
                            bias=b2t[:, n2t : n2t + 1],
                        )
                        nc.scalar.dma_start(
                            out2T[
                                n2t * 128 : (n2t + 1) * 128, mb * 512 : (mb + 1) * 512
                            ],
                            ot[:],
                        )

    _elide_redundant_ldweights(nc)
    _split_excess_waits(nc)
    return nc


def _prep_inputs(x, adj, W1, b1, W2, b2):
    bf = ml_dtypes.bfloat16
    w1b = W1.astype(bf)
    w2b = W2.astype(bf)
    b1T = np.ascontiguousarray(b1.reshape(HID // 128, 128).T).astype(np.float32)
    b2T = np.ascontiguousarray(b2.reshape(OUT // 128, 128).T).astype(np.float32)
    in_maps = []
    for c in range(NCORES):
        rows = slice(c * SH, (c + 1) * SH)
        in_maps.append(
            {
                "adjT": adj[rows, :].T.astype(bf),
                "xT": x[rows, :].T.astype(bf),
                "w1": w1b,
                "w2": w2b,
                "b1T": b1T,
                "b2T": b2T,
            }
        )
    return in_maps


def _run(inputs, trace=False):
    global _built
    if _built is None:
        _built = build()
    in_maps = _prep_inputs(**inputs)
    r = run_bass_kernel_spmd(_built, in_maps, list(range(NCORES)), trace=trace)
    out = np.empty([N, OUT], np.float32)
    for c in range(NCORES):
        out[c * SH : (c + 1) * SH, :] = r.results[c]["out2T"].T
    return out, r


def kernel(x, adj, W1, b1, W2, b2):
    out, _ = _run(dict(x=x, adj=adj, W1=W1, b1=b1, W2=W2, b2=b2))
    return out



# revision 13
# speedup vs baseline: 3.1521x; 3.1521x over previous
"""Trainium2 Bass kernel for a 2-layer dense GCN (NodeEncoder).

    out = adj @ relu(adj @ (x@W1) + b1) @ W2 + b2
    N=16384, F_IN=512, HID=1024, OUT=256, adj dense [N, N] fp32.

Algorithm (reassociated to nearly halve layer-1 FLOPs and drop the big
s1 AllGather):  relu(adj @ (x@W1)) == relu((adj@x) @ W1), so per core
(adj row-partitioned, 2048 rows each):

  P1:    yT_c   = x8^T @ adjN8_c^T                     [512, 2048]  (= N*y^T)
  small: hT_c   = relu(yT_c^T @ (W1/N) + b1)^T         [1024, 2048] bf16
         s2_c   = h_c @ W2                             [2048, 256]
         quantized to fp8 * 256 for the gather.
  AG:    s2q    = AllGather(s2q_c)  (4 chunks of 128KB, overlapped)
  P2:    out2T_c = (adjN8_c @ s2q)^T / (N*256) + b2    [256, 2048] fp32

Big matmuls run in fp8-e4m3 DoubleRow (K=256/instr); small ones bf16.
Simulated end-to-end rel err ~1.55e-2 vs fp32 reference (tol 2e-2),
dominated by the fp8 quantization of x.
"""

import numpy as np
import ml_dtypes

import concourse.bass as bass
import concourse.mybir as mybir
import concourse.tile as tile
from concourse.bass_utils import run_bass_kernel_spmd
from concourse.tile_sem_assignment import N_PROCS
from concourse.vector_clock import ScopedClock, VectorClock

# ---------------------------------------------------------------------------
# Workaround: the walrus build in this container caps the number of sync-wait
# commands per instruction at ONE.  Tile's kernel-tail drain aggregates one
# wait per logical processor; split it into a chain of single-wait drains.
# Excess waits on regular instructions are hoisted onto no-ops.
# ---------------------------------------------------------------------------


def _drain_and_barrier_split(self, tick_clock, wait_clock):
    gc = tick_clock.global_clock
    for p in range(N_PROCS):
        partial = VectorClock([gc[q] if q == p else 0 for q in range(N_PROCS)])
        d = self.nc.sync.drain()
        wait_clock.add_sem_waits(d.ins, ScopedClock({None: partial}))
    self.nc.sync.drain()

    self.nc.all_engine_barrier()
    assert self.sems is not None
    popped = self.nc._tile_sem_poison_stack.pop()
    assert popped is self._sem_poison
    self.nc.clear_and_free_semaphores(list(self.sems.allocated().values()))
    self.nc.all_engine_barrier()


tile.TileContext._drain_and_barrier = _drain_and_barrier_split

_MAX_WAITS = 1


def _split_excess_waits(nc):
    ctr = 0
    for f in nc.m.functions:
        for bb in f.blocks:
            out = []
            changed = False
            for inst in bb.instructions:
                si = inst.sync_info
                waits = list(si.on_wait) if si is not None and si.on_wait else []
                if len(waits) > _MAX_WAITS:
                    changed = True
                    keep, excess = waits[: _MAX_WAITS], waits[_MAX_WAITS :]
                    for i in range(0, len(excess), _MAX_WAITS):
                        ctr += 1
                        nop = mybir.InstNoOp(name=f"I-waitnop-{ctr}")
                        nop.engine = inst.engine
                        nop.sync_info = mybir.SyncInfo(
                            on_wait=excess[i : i + _MAX_WAITS], on_update=[]
                        )
                        out.append(nop)
                    si.on_wait = keep
                out.append(inst)
            if changed:
                bb.instructions = out
    return ctr


def _elide_redundant_ldweights(nc):
    """Drop an InstLdweights that reloads the same weights AP as the previous
    surviving one with only plain matmuls/no-ops in between (the PE keeps the
    stationary operand across matmuls; walrus emits one LDWEIGHTS per MATMUL)."""
    n_elided = 0
    for f in nc.m.functions:
        for bb in f.blocks:
            out = []
            last_w = None
            changed = False
            for inst in bb.instructions:
                nm = type(inst).__name__
                if nm == "InstLdweights":
                    si = inst.sync_info
                    clean = not (si and (si.on_wait or si.on_update))
                    w = repr(inst.ins[0])
                    if clean and last_w == w:
                        n_elided += 1
                        changed = True
                        continue
                    last_w = w if clean else None
                elif nm == "InstMatmult":
                    if getattr(inst, "is_transpose", False):
                        last_w = None
                elif nm == "InstNoOp":
                    pass
                else:
                    last_w = None
                out.append(inst)
            if changed:
                bb.instructions = out
    return n_elided


NCORES = 8
N = 16384
SH = N // NCORES  # 2048 adj rows per core
F = 512
HID = 1024
OUT = 256
S2SCALE = 256.0  # s2 is gathered as fp8 of 256*s2

BF16 = mybir.dt.bfloat16
F32 = mybir.dt.float32
FP8 = mybir.dt.float8e4
DR = mybir.MatmulPerfMode.DoubleRow

_built = None


def build():
    nc = bass.Bass()

    # adjU row r = kb*128 + p (k = kb*256 + kk*128 + p global col of adjT_c),
    # col = kk*2048 + i (i = local row of the adj shard), values N*adj in fp8.
    # P1 reads the 1024-wide i-half slices, P2 reads full rows.
    adjU = nc.declare_dram_parameter("adjU", [8192, 2 * SH], FP8, isOutput=False)
    # xP row = kb*128 + p, col = kk*512 + j
    xP = nc.declare_dram_parameter("xP", [8192, 2 * F], FP8, isOutput=False)
    w1n = nc.declare_dram_parameter("w1n", [F, HID], BF16, isOutput=False)  # W1/N
    w2 = nc.declare_dram_parameter("w2", [HID, OUT], BF16, isOutput=False)
    b1T = nc.declare_dram_parameter("b1T", [128, HID // 128], F32, isOutput=False)
    b2T = nc.declare_dram_parameter("b2T", [128, OUT // 128], F32, isOutput=False)
    out2T = nc.declare_dram_parameter("out2T", [OUT, SH], F32, isOutput=True)

    rg = [list(range(NCORES))]

    def allgather(inp, outp):
        return nc.gpsimd.collective_compute(
            "AllGather",
            mybir.AluOpType.bypass,
            replica_groups=rg,
            ins=[inp.opt()],
            outs=[outp.opt()],
        )

    with tile.TileContext(nc) as tc:
        with (
            tc.tile_pool(name="const", bufs=1) as constp,
            tc.tile_pool(name="psum", bufs=8, space="PSUM") as psum,
            tc.tile_pool(name="dram", bufs=1, space="DRAM") as dram,
            tc.tile_pool(name="adj", bufs=4) as adjp,
            tc.tile_pool(name="small", bufs=4) as smallp,
        ):
            # ---- constants / resident tensors ----
            w1t = constp.tile([128, 4, HID], BF16)  # [j%128, jj, hid]
            nc.sync.dma_start(w1t[:], w1n[:].rearrange("(jj p) h -> p jj h", p=128))
            w2t = constp.tile([128, 8, OUT], BF16)  # [hid%128, hh, j2]
            nc.sync.dma_start(w2t[:], w2[:].rearrange("(hh p) o -> p hh o", p=128))
            b1t = constp.tile([128, 8], F32)
            nc.sync.dma_start(b1t[:], b1T[:])
            b2t = constp.tile([128, 2], F32)
            nc.sync.dma_start(b2t[:], b2T[:])
            # all of x resident: [p, kb, kk, j]
            xt = constp.tile([128, 64, 2, F], FP8)
            nc.sync.dma_start(
                xt[:], xP[:].rearrange("(kb p) (kk j) -> p kb kk j", p=128, kk=2)
            )

            # results kept in SBUF
            yT = constp.tile([128, 4, SH], BF16)  # [j%128, jj, i] = N*y
            hT = constp.tile([128, 8, SH], BF16)  # [hid%128, hh, i]

            # AllGather staging: chunk q covers local rows [512q, 512q+512)
            # laid out [p, kbl, kk, j2] (row = kbl*256 + kk*128 + p).
            ag_in = [dram.tile([128, 1024], FP8, name=f"agi{q}") for q in range(4)]
            ag_out = [
                dram.tile([NCORES * 128, 1024], FP8, addr_space="Shared", name=f"ago{q}")
                for q in range(4)
            ]

            adjU_r = adjU[:].rearrange("(kb p) (kk i) -> p kb kk i", p=128, kk=2)

            for H in range(2):
                # ---- P1 half H: psY[j, i-1024-half] += x8^T adjC8 ----
                psY = [
                    psum.tile([128, 512], F32, tag="ps", name=f"psY{H}{t}")
                    for t in range(8)
                ]  # tile t = (jj, b): jj*2 + b; holds i-chunks 2b, 2b+1
                for kb in range(64):
                    at = adjp.tile([128, 2, 1024], FP8, tag="adjt", name=f"a1_{H}_{kb}")
                    nc.sync.dma_start(
                        at[:], adjU_r[:, kb, :, H * 1024 : (H + 1) * 1024]
                    )
                    for jj in range(4):
                        lhs = xt[:, kb, :, jj * 128 : (jj + 1) * 128]
                        for c in range(4):  # i-chunk of 256 within the half
                            nc.tensor.matmul(
                                psY[jj * 2 + c // 2][:, (c % 2) * 256 : (c % 2) * 256 + 256],
                                lhs,
                                at[:, :, c * 256 : (c + 1) * 256],
                                start=(kb == 0 and c % 2 == 0),
                                stop=(kb == 63 and c % 2 == 1),
                                perf_mode=DR,
                            )
                # drain psY -> yT (bf16) on the vector engine (ACT is busier)
                for jj in range(4):
                    for b in range(2):
                        nc.vector.tensor_copy(
                            yT[:, jj, H * 1024 + b * 512 : H * 1024 + b * 512 + 512],
                            psY[jj * 2 + b][:],
                        )

                # ---- supportT + relu: hT = relu(W1n^T yT + b1) ----
                for qq in range(2):  # i-quarter (512) within the half
                    i0 = H * 1024 + qq * 512
                    psS = [
                        psum.tile([128, 512], F32, tag="ps", name=f"psS{H}{qq}{hh}")
                        for hh in range(8)
                    ]
                    for hh in range(8):
                        for jj in range(4):
                            nc.tensor.matmul(
                                psS[hh][:],
                                w1t[:, jj, hh * 128 : (hh + 1) * 128],
                                yT[:, jj, i0 : i0 + 512],
                                start=(jj == 0),
                                stop=(jj == 3),
                            )
                    for hh in range(8):
                        nc.scalar.activation(
                            hT[:, hh, i0 : i0 + 512],
                            psS[hh][:],
                            mybir.ActivationFunctionType.Relu,
                            bias=b1t[:, hh : hh + 1],
                        )

                # ---- s2 = h @ W2, quantized fp8*256, staged for AG ----
                # psum/AG column order (j2t, kk, jp) so P2's stationary load
                # is a plain contiguous copy.
                for qq in range(2):
                    q = H * 2 + qq  # global chunk id
                    for kbl in range(2):
                        ps2 = psum.tile(
                            [128, 2, 2, 128], F32, tag="ps", name=f"ps2{q}{kbl}"
                        )
                        for kk in range(2):
                            i0 = q * 512 + kbl * 256 + kk * 128
                            for hh in range(8):
                                nc.tensor.matmul(
                                    ps2[:, :, kk, :],
                                    hT[:, hh, i0 : i0 + 128],
                                    w2t[:, hh, :],
                                    start=(hh == 0 and kk == 0),
                                    stop=(hh == 7 and kk == 1),
                                )
                        s2q = smallp.tile([128, 512], FP8, tag="s2q", bufs=2)
                        nc.scalar.activation(
                            s2q[:],
                            ps2[:].rearrange("p a b j -> p (a b j)"),
                            mybir.ActivationFunctionType.Copy,
                            scale=S2SCALE,
                        )
                        nc.scalar.dma_start(
                            ag_in[q][:, kbl * 512 : kbl * 512 + 512], s2q[:]
                        )
                    allgather(ag_in[q], ag_out[q])

            # ---- P2: out2T = (adjU8 @ s2q)^T / (N*256) + b2 ----
            psD = [
                psum.tile([128, 512], F32, tag="ps", name=f"psD{t}") for t in range(8)
            ]  # tile t = (j2, cb): j2*4 + cb; holds i-chunks 2cb, 2cb+1
            kb_order = [
                8 * c + 2 * q + t for q in range(4) for c in range(NCORES) for t in range(2)
            ]
            for ki, kb in enumerate(kb_order):
                c, rem = kb // 8, kb % 8
                q, t = rem // 2, rem % 2
                at = adjp.tile([128, 2, 2048], FP8, tag="adjt", name=f"a2_{kb}")
                nc.sync.dma_start(at[:], adjU_r[:, kb])
                st = smallp.tile([128, 2, 2, 128], FP8, tag="st", bufs=4, name=f"st{kb}")
                # ag_out rows c*128+p, cols t*512 + j2t*256 + kk*128 + jp
                nc.sync.dma_start(
                    st[:].rearrange("p a b j -> p (a b j)"),
                    ag_out[q][c * 128 : (c + 1) * 128, t * 512 : (t + 1) * 512],
                )
                for j2 in range(2):
                    lhs = st[:, j2]
                    for c8 in range(8):
                        nc.tensor.matmul(
                            psD[j2 * 4 + c8 // 2][:, (c8 % 2) * 256 : (c8 % 2) * 256 + 256],
                            lhs,
                            at[:, :, c8 * 256 : (c8 + 1) * 256],
                            start=(ki == 0 and c8 % 2 == 0),
                            stop=(ki == 63 and c8 % 2 == 1),
                            perf_mode=DR,
                        )
            for j2 in range(2):
                for cb in range(4):
                    ot = smallp.tile([128, 512], F32, tag="ot", bufs=2)
                    nc.scalar.activation(
                        ot[:],
                        psD[j2 * 4 + cb][:],
                        mybir.ActivationFunctionType.Identity,
                        bias=b2t[:, j2 : j2 + 1],
                        scale=1.0 / (N * S2SCALE),
                    )
                    nc.scalar.dma_start(
                        out2T[j2 * 128 : (j2 + 1) * 128, cb * 512 : (cb + 1) * 512],
                        ot[:],
                    )

    _elide_redundant_ldweights(nc)
    _split_excess_waits(nc)
    return nc


def _prep_inputs(x, adj, W1, b1, W2, b2):
    bf = ml_dtypes.bfloat16
    f8 = ml_dtypes.float8_e4m3fn

    u = adj * np.float32(N)  # exact: adj was u/N with N a power of two
    u8 = u.astype(f8)
    x8 = x.astype(f8)
    b1T = np.ascontiguousarray(b1.reshape(HID // 128, 128).T).astype(np.float32)
    b2T = np.ascontiguousarray(b2.reshape(OUT // 128, 128).T).astype(np.float32)
    w1n = (W1 / np.float32(N)).astype(bf)
    w2b = W2.astype(bf)
    # xP[kb*128+p, kk*512+j] = x8[kb*256+kk*128+p, j]
    xP = np.ascontiguousarray(
        x8.reshape(64, 2, 128, F).transpose(0, 2, 1, 3).reshape(8192, 2 * F)
    )

    def adj_layout(a8, rows):
        # out[kb*128+p, kk*2048+i] = a8[rows][i, kb*256+kk*128+p]
        blk = a8[rows, :].reshape(SH, 64, 2, 128)  # [i, kb, kk, p]
        return np.ascontiguousarray(
            blk.transpose(1, 3, 2, 0).reshape(8192, 2 * SH)
        )

    in_maps = []
    for c in range(NCORES):
        rows = slice(c * SH, (c + 1) * SH)
        in_maps.append(
            {
                "adjU": adj_layout(u8, rows),
                "xP": xP,
                "w1n": w1n,
                "w2": w2b,
                "b1T": b1T,
                "b2T": b2T,
            }
        )
    return in_maps


def _run(inputs, trace=False):
    global _built
    if _built is None:
        _built = build()
    in_maps = _prep_inputs(**inputs)
    r = run_bass_kernel_spmd(_built, in_maps, list(range(NCORES)), trace=trace)
    out = np.empty([N, OUT], np.float32)
    for c in range(NCORES):
        out[c * SH : (c + 1) * SH, :] = r.results[c]["out2T"].T
    return out, r


def kernel(x, adj, W1, b1, W2, b2):
    out, _ = _run(dict(x=x, adj=adj, W1=W1, b1=b1, W2=W2, b2=b2))
    return out


# revision 18
# speedup vs baseline: 3.3107x; 1.0503x over previous
"""Trainium2 Bass kernel for a 2-layer dense GCN (NodeEncoder).

    out = adj @ relu(adj @ (x@W1) + b1) @ W2 + b2
    N=16384, F_IN=512, HID=1024, OUT=256, adj dense [N, N] fp32.

Algorithm (reassociated to nearly halve layer-1 FLOPs and drop the big
s1 AllGather):  relu(adj @ (x@W1)) == relu((adj@x) @ W1), so per core
(adj row-partitioned, 2048 rows each):

  P1:    yT_c   = x8^T @ adjN8_c^T                     [512, 2048]  (= N*y^T)
  small: hT_c   = relu(yT_c^T @ (W1/N) + b1)^T         [1024, 2048] bf16
         s2_c   = h_c @ W2                             [2048, 256]
         quantized to fp8 * 256 for the gather.
  AG:    s2q    = AllGather(s2q_c)  (4 chunks of 128KB, overlapped)
  P2:    out2T_c = (adjN8_c @ s2q)^T / (N*256) + b2    [256, 2048] fp32

Big matmuls run in fp8-e4m3 DoubleRow (K=256/instr); small ones bf16.
Simulated end-to-end rel err ~1.55e-2 vs fp32 reference (tol 2e-2),
dominated by the fp8 quantization of x.
"""

import numpy as np
import ml_dtypes

import concourse.bass as bass
import concourse.mybir as mybir
import concourse.tile as tile
from concourse.bass_utils import run_bass_kernel_spmd
from concourse.tile_sem_assignment import N_PROCS
from concourse.vector_clock import ScopedClock, VectorClock

# ---------------------------------------------------------------------------
# Workaround: the walrus build in this container caps the number of sync-wait
# commands per instruction at ONE.  Tile's kernel-tail drain aggregates one
# wait per logical processor; split it into a chain of single-wait drains.
# Excess waits on regular instructions are hoisted onto no-ops.
# ---------------------------------------------------------------------------


def _drain_and_barrier_split(self, tick_clock, wait_clock):
    gc = tick_clock.global_clock
    for p in range(N_PROCS):
        partial = VectorClock([gc[q] if q == p else 0 for q in range(N_PROCS)])
        d = self.nc.sync.drain()
        wait_clock.add_sem_waits(d.ins, ScopedClock({None: partial}))
    self.nc.sync.drain()

    self.nc.all_engine_barrier()
    assert self.sems is not None
    popped = self.nc._tile_sem_poison_stack.pop()
    assert popped is self._sem_poison
    self.nc.clear_and_free_semaphores(list(self.sems.allocated().values()))
    self.nc.all_engine_barrier()


tile.TileContext._drain_and_barrier = _drain_and_barrier_split

_MAX_WAITS = 1


def _split_excess_waits(nc):
    ctr = 0
    for f in nc.m.functions:
        for bb in f.blocks:
            out = []
            changed = False
            for inst in bb.instructions:
                si = inst.sync_info
                waits = list(si.on_wait) if si is not None and si.on_wait else []
                if len(waits) > _MAX_WAITS:
                    changed = True
                    keep, excess = waits[: _MAX_WAITS], waits[_MAX_WAITS :]
                    for i in range(0, len(excess), _MAX_WAITS):
                        ctr += 1
                        nop = mybir.InstNoOp(name=f"I-waitnop-{ctr}")
                        nop.engine = inst.engine
                        nop.sync_info = mybir.SyncInfo(
                            on_wait=excess[i : i + _MAX_WAITS], on_update=[]
                        )
                        out.append(nop)
                    si.on_wait = keep
                out.append(inst)
            if changed:
                bb.instructions = out
    return ctr


def _elide_redundant_ldweights(nc):
    """Drop an InstLdweights that reloads the same weights AP as the previous
    surviving one with only plain matmuls/no-ops in between (the PE keeps the
    stationary operand across matmuls; walrus emits one LDWEIGHTS per MATMUL)."""
    n_elided = 0
    for f in nc.m.functions:
        for bb in f.blocks:
            out = []
            last_w = None
            changed = False
            for inst in bb.instructions:
                nm = type(inst).__name__
                if nm == "InstLdweights":
                    si = inst.sync_info
                    clean = not (si and (si.on_wait or si.on_update))
                    w = repr(inst.ins[0])
                    if clean and last_w == w:
                        n_elided += 1
                        changed = True
                        continue
                    last_w = w if clean else None
                elif nm == "InstMatmult":
                    if getattr(inst, "is_transpose", False):
                        last_w = None
                elif nm == "InstNoOp":
                    pass
                else:
                    last_w = None
                out.append(inst)
            if changed:
                bb.instructions = out
    return n_elided


NCORES = 8
N = 16384
SH = N // NCORES  # 2048 adj rows per core
F = 512
HID = 1024
OUT = 256
S2SCALE = 256.0  # s2 is gathered as fp8 of 256*s2

BF16 = mybir.dt.bfloat16
F32 = mybir.dt.float32
FP8 = mybir.dt.float8e4
DR = mybir.MatmulPerfMode.DoubleRow

_built = None


def build():
    nc = bass.Bass()

    # adjU row r = kb*128 + p (k = kb*256 + kk*128 + p global col of adjT_c),
    # col = kk*2048 + i (i = local row of the adj shard), values N*adj in fp8.
    # P1 reads the 1024-wide i-half slices, P2 reads full rows.
    adjU = nc.declare_dram_parameter("adjU", [8192, 2 * SH], FP8, isOutput=False)
    # xP row = kb*128 + p, col = kk*512 + j
    xP = nc.declare_dram_parameter("xP", [8192, 2 * F], FP8, isOutput=False)
    w1n = nc.declare_dram_parameter("w1n", [F, HID], BF16, isOutput=False)  # W1/N
    w2 = nc.declare_dram_parameter("w2", [HID, OUT], BF16, isOutput=False)
    b1T = nc.declare_dram_parameter("b1T", [128, HID // 128], F32, isOutput=False)
    b2T = nc.declare_dram_parameter("b2T", [128, OUT // 128], F32, isOutput=False)
    out2T = nc.declare_dram_parameter("out2T", [OUT, SH], F32, isOutput=True)

    rg = [list(range(NCORES))]

    def allgather(inp, outp):
        return nc.gpsimd.collective_compute(
            "AllGather",
            mybir.AluOpType.bypass,
            replica_groups=rg,
            ins=[inp.opt()],
            outs=[outp.opt()],
        )

    with tile.TileContext(nc) as tc:
        with (
            tc.tile_pool(name="const", bufs=1) as constp,
            tc.tile_pool(name="psum", bufs=8, space="PSUM") as psum,
            tc.tile_pool(name="dram", bufs=1, space="DRAM") as dram,
            tc.tile_pool(name="adj", bufs=4) as adjp,
            tc.tile_pool(name="small", bufs=4) as smallp,
        ):
            # ---- constants / resident tensors ----
            w1t = constp.tile([128, 4, HID], BF16)  # [j%128, jj, hid]
            nc.sync.dma_start(w1t[:], w1n[:].rearrange("(jj p) h -> p jj h", p=128))
            w2t = constp.tile([128, 8, OUT], BF16)  # [hid%128, hh, j2]
            nc.sync.dma_start(w2t[:], w2[:].rearrange("(hh p) o -> p hh o", p=128))
            b1t = constp.tile([128, 8], F32)
            nc.sync.dma_start(b1t[:], b1T[:])
            b2t = constp.tile([128, 2], F32)
            nc.sync.dma_start(b2t[:], b2T[:])
            # all of x resident: [p, kb, kk, j]; loaded in 8 chunks interleaved
            # with the first P1 adj loads so the first matmul starts early
            xt = constp.tile([128, 64, 2, F], FP8)
            xP_r = xP[:].rearrange("(kb p) (kk j) -> p kb kk j", p=128, kk=2)

            # results kept in SBUF
            yT = constp.tile([128, 4, SH], BF16)  # [j%128, jj, i] = N*y
            hT = constp.tile([128, 8, SH], BF16)  # [hid%128, hh, i]

            # AllGather staging: chunk q covers local rows [512q, 512q+512)
            # laid out [p, kbl, kk, j2] (row = kbl*256 + kk*128 + p).
            ag_in = [dram.tile([128, 1024], FP8, name=f"agi{q}") for q in range(4)]
            ag_out = [
                dram.tile([NCORES * 128, 1024], FP8, addr_space="Shared", name=f"ago{q}")
                for q in range(4)
            ]

            adjU_r = adjU[:].rearrange("(kb p) (kk i) -> p kb kk i", p=128, kk=2)

            for H in range(2):
                # ---- P1 half H: psY[j, i-1024-half] += x8^T adjC8 ----
                psY = [
                    psum.tile([128, 512], F32, tag="ps", name=f"psY{H}{t}")
                    for t in range(8)
                ]  # tile t = (jj, b): jj*2 + b; holds i-chunks 2b, 2b+1
                for kb in range(64):
                    if H == 0 and kb % 8 == 0:
                        g = kb // 8
                        nc.sync.dma_start(
                            xt[:, g * 8 : g * 8 + 8], xP_r[:, g * 8 : g * 8 + 8]
                        )
                    at = adjp.tile([128, 2, 1024], FP8, tag="adjt", name=f"a1_{H}_{kb}")
                    nc.sync.dma_start(
                        at[:], adjU_r[:, kb, :, H * 1024 : (H + 1) * 1024]
                    )
                    for jj in range(4):
                        lhs = xt[:, kb, :, jj * 128 : (jj + 1) * 128]
                        for c in range(4):  # i-chunk of 256 within the half
                            nc.tensor.matmul(
                                psY[jj * 2 + c // 2][:, (c % 2) * 256 : (c % 2) * 256 + 256],
                                lhs,
                                at[:, :, c * 256 : (c + 1) * 256],
                                start=(kb == 0 and c % 2 == 0),
                                stop=(kb == 63 and c % 2 == 1),
                                perf_mode=DR,
                            )
                # drain psY -> yT (bf16) on the vector engine (ACT is busier)
                for jj in range(4):
                    for b in range(2):
                        nc.vector.tensor_copy(
                            yT[:, jj, H * 1024 + b * 512 : H * 1024 + b * 512 + 512],
                            psY[jj * 2 + b][:],
                        )

                # ---- supportT + relu: hT = relu(W1n^T yT + b1) ----
                # hh-groups of 4 with i-width 1024: each stationary W1 block
                # feeds two 512-wide matmuls, halving LDWEIGHTS count.
                for hg in range(2):
                    i0 = H * 1024
                    psS = [
                        psum.tile([128, 512], F32, tag="ps", name=f"psS{H}{hg}{t}")
                        for t in range(8)
                    ]  # t = hh4*2 + qq
                    for hh4 in range(4):
                        hh = hg * 4 + hh4
                        for jj in range(4):
                            for qq in range(2):
                                nc.tensor.matmul(
                                    psS[hh4 * 2 + qq][:],
                                    w1t[:, jj, hh * 128 : (hh + 1) * 128],
                                    yT[:, jj, i0 + qq * 512 : i0 + qq * 512 + 512],
                                    start=(jj == 0),
                                    stop=(jj == 3),
                                )
                    for hh4 in range(4):
                        hh = hg * 4 + hh4
                        for qq in range(2):
                            nc.scalar.activation(
                                hT[:, hh, i0 + qq * 512 : i0 + qq * 512 + 512],
                                psS[hh4 * 2 + qq][:],
                                mybir.ActivationFunctionType.Relu,
                                bias=b1t[:, hh : hh + 1],
                            )

                # ---- s2 = h @ W2, quantized fp8*256, staged for AG ----
                # psum/AG column order (j2t, kk, jp) so P2's stationary load
                # is a plain contiguous copy.
                for qq in range(2):
                    q = H * 2 + qq  # global chunk id
                    for kbl in range(2):
                        ps2 = psum.tile(
                            [128, 2, 2, 128], F32, tag="ps", name=f"ps2{q}{kbl}"
                        )
                        for kk in range(2):
                            i0 = q * 512 + kbl * 256 + kk * 128
                            for hh in range(8):
                                nc.tensor.matmul(
                                    ps2[:, :, kk, :],
                                    hT[:, hh, i0 : i0 + 128],
                                    w2t[:, hh, :],
                                    start=(hh == 0 and kk == 0),
                                    stop=(hh == 7 and kk == 1),
                                )
                        s2q = smallp.tile([128, 512], FP8, tag="s2q", bufs=2)
                        nc.scalar.activation(
                            s2q[:],
                            ps2[:].rearrange("p a b j -> p (a b j)"),
                            mybir.ActivationFunctionType.Copy,
                            scale=S2SCALE,
                        )
                        nc.scalar.dma_start(
                            ag_in[q][:, kbl * 512 : kbl * 512 + 512], s2q[:]
                        )
                    allgather(ag_in[q], ag_out[q])

            # ---- P2: out2T = (adjU8 @ s2q)^T / (N*256) + b2 ----
            psD = [
                psum.tile([128, 512], F32, tag="ps", name=f"psD{t}") for t in range(8)
            ]  # tile t = (j2, cb): j2*4 + cb; holds i-chunks 2cb, 2cb+1
            kb_order = [
                8 * c + 2 * q + t for q in range(4) for c in range(NCORES) for t in range(2)
            ]
            for ki, kb in enumerate(kb_order):
                c, rem = kb // 8, kb % 8
                q, t = rem // 2, rem % 2
                at = adjp.tile([128, 2, 2048], FP8, tag="adjt", name=f"a2_{kb}")
                nc.sync.dma_start(at[:], adjU_r[:, kb])
                st = smallp.tile([128, 2, 2, 128], FP8, tag="st", bufs=4, name=f"st{kb}")
                # ag_out rows c*128+p, cols t*512 + j2t*256 + kk*128 + jp
                nc.sync.dma_start(
                    st[:].rearrange("p a b j -> p (a b j)"),
                    ag_out[q][c * 128 : (c + 1) * 128, t * 512 : (t + 1) * 512],
                )
                for j2 in range(2):
                    lhs = st[:, j2]
                    for c8 in range(8):
                        nc.tensor.matmul(
                            psD[j2 * 4 + c8 // 2][:, (c8 % 2) * 256 : (c8 % 2) * 256 + 256],
                            lhs,
                            at[:, :, c8 * 256 : (c8 + 1) * 256],
                            start=(ki == 0 and c8 % 2 == 0),
                            stop=(ki == 63 and c8 % 2 == 1),
                            perf_mode=DR,
                        )
            # final drain split across ACT and DVE so the tail is ~2x shorter
            for j2 in range(2):
                for cb in range(4):
                    ot = smallp.tile([128, 512], F32, tag="ot", bufs=4)
                    if cb % 2 == 0:
                        nc.scalar.activation(
                            ot[:],
                            psD[j2 * 4 + cb][:],
                            mybir.ActivationFunctionType.Identity,
                            bias=b2t[:, j2 : j2 + 1],
                            scale=1.0 / (N * S2SCALE),
                        )
                        nc.scalar.dma_start(
                            out2T[j2 * 128 : (j2 + 1) * 128, cb * 512 : (cb + 1) * 512],
                            ot[:],
                        )
                    else:
                        nc.vector.tensor_scalar(
                            ot[:],
                            psD[j2 * 4 + cb][:],
                            1.0 / (N * S2SCALE),
                            b2t[:, j2 : j2 + 1],
                            op0=mybir.AluOpType.mult,
                            op1=mybir.AluOpType.add,
                        )
                        nc.gpsimd.dma_start(
                            out2T[j2 * 128 : (j2 + 1) * 128, cb * 512 : (cb + 1) * 512],
                            ot[:],
                        )

    _elide_redundant_ldweights(nc)
    _split_excess_waits(nc)
    return nc


def _prep_inputs(x, adj, W1, b1, W2, b2):
    bf = ml_dtypes.bfloat16
    f8 = ml_dtypes.float8_e4m3fn

    u = adj * np.float32(N)  # exact: adj was u/N with N a power of two
    u8 = u.astype(f8)
    x8 = x.astype(f8)
    b1T = np.ascontiguousarray(b1.reshape(HID // 128, 128).T).astype(np.float32)
    b2T = np.ascontiguousarray(b2.reshape(OUT // 128, 128).T).astype(np.float32)
    w1n = (W1 / np.float32(N)).astype(bf)
    w2b = W2.astype(bf)
    # xP[kb*128+p, kk*512+j] = x8[kb*256+kk*128+p, j]
    xP = np.ascontiguousarray(
        x8.reshape(64, 2, 128, F).transpose(0, 2, 1, 3).reshape(8192, 2 * F)
    )

    def adj_layout(a8, rows):
        # out[kb*128+p, kk*2048+i] = a8[rows][i, kb*256+kk*128+p]
        blk = a8[rows, :].reshape(SH, 64, 2, 128)  # [i, kb, kk, p]
        return np.ascontiguousarray(
            blk.transpose(1, 3, 2, 0).reshape(8192, 2 * SH)
        )

    in_maps = []
    for c in range(NCORES):
        rows = slice(c * SH, (c + 1) * SH)
        in_maps.append(
            {
                "adjU": adj_layout(u8, rows),
                "xP": xP,
                "w1n": w1n,
                "w2": w2b,
                "b1T": b1T,
                "b2T": b2T,
            }
        )
    return in_maps


def _run(inputs, trace=False):
    global _built
    if _built is None:
        _built = build()
    in_maps = _prep_inputs(**inputs)
    r = run_bass_kernel_spmd(_built, in_maps, list(range(NCORES)), trace=trace)
    out = np.empty([N, OUT], np.float32)
    for c in range(NCORES):
        out[c * SH : (c + 1) * SH, :] = r.results[c]["out2T"].T
    return out, r


def kernel(x, adj, W1, b1, W2, b2):
    out, _ = _run(dict(x=x, adj=adj, W1=W1, b1=b1, W2=W2, b2=b2))
    return out


# revision 23
# speedup vs baseline: 3.4108x; 1.0302x over previous
"""Trainium2 Bass kernel for a 2-layer dense GCN (NodeEncoder).

    out = adj @ relu(adj @ (x@W1) + b1) @ W2 + b2
    N=16384, F_IN=512, HID=1024, OUT=256, adj dense [N, N] fp32.

Algorithm (reassociated to nearly halve layer-1 FLOPs and drop the big
s1 AllGather):  relu(adj @ (x@W1)) == relu((adj@x) @ W1), so per core
(adj row-partitioned, 2048 rows each):

  P1:    yT_c   = x8^T @ adjN8_c^T                     [512, 2048]  (= N*y^T)
  small: hT_c   = relu(yT_c^T @ (W1/N) + b1)^T         [1024, 2048] bf16
         s2_c   = h_c @ W2                             [2048, 256]
         quantized to fp8 * 256 for the gather.
  AG:    s2q    = AllGather(s2q_c)  (4 chunks of 128KB, overlapped)
  P2:    out2T_c = (adjN8_c @ s2q)^T / (N*256) + b2    [256, 2048] fp32

Big matmuls run in fp8-e4m3 DoubleRow (K=256/instr); small ones bf16.
Simulated end-to-end rel err ~1.55e-2 vs fp32 reference (tol 2e-2),
dominated by the fp8 quantization of x.
"""

import numpy as np
import ml_dtypes

import concourse.bass as bass
import concourse.mybir as mybir
import concourse.tile as tile
from concourse.bass_utils import run_bass_kernel_spmd
from concourse.tile_sem_assignment import N_PROCS
from concourse.vector_clock import ScopedClock, VectorClock

# ---------------------------------------------------------------------------
# Workaround: the walrus build in this container caps the number of sync-wait
# commands per instruction at ONE.  Tile's kernel-tail drain aggregates one
# wait per logical processor; split it into a chain of single-wait drains.
# Excess waits on regular instructions are hoisted onto no-ops.
# ---------------------------------------------------------------------------


def _drain_and_barrier_split(self, tick_clock, wait_clock):
    gc = tick_clock.global_clock
    for p in range(N_PROCS):
        partial = VectorClock([gc[q] if q == p else 0 for q in range(N_PROCS)])
        d = self.nc.sync.drain()
        wait_clock.add_sem_waits(d.ins, ScopedClock({None: partial}))
    self.nc.sync.drain()

    self.nc.all_engine_barrier()
    assert self.sems is not None
    popped = self.nc._tile_sem_poison_stack.pop()
    assert popped is self._sem_poison
    self.nc.clear_and_free_semaphores(list(self.sems.allocated().values()))
    self.nc.all_engine_barrier()


tile.TileContext._drain_and_barrier = _drain_and_barrier_split

_MAX_WAITS = 1


def _split_excess_waits(nc):
    ctr = 0
    for f in nc.m.functions:
        for bb in f.blocks:
            out = []
            changed = False
            for inst in bb.instructions:
                si = inst.sync_info
                waits = list(si.on_wait) if si is not None and si.on_wait else []
                if len(waits) > _MAX_WAITS:
                    changed = True
                    keep, excess = waits[: _MAX_WAITS], waits[_MAX_WAITS :]
                    for i in range(0, len(excess), _MAX_WAITS):
                        ctr += 1
                        nop = mybir.InstNoOp(name=f"I-waitnop-{ctr}")
                        nop.engine = inst.engine
                        nop.sync_info = mybir.SyncInfo(
                            on_wait=excess[i : i + _MAX_WAITS], on_update=[]
                        )
                        out.append(nop)
                    si.on_wait = keep
                out.append(inst)
            if changed:
                bb.instructions = out
    return ctr


def _elide_redundant_ldweights(nc):
    """Drop an InstLdweights that reloads the same weights AP as the previous
    surviving one with only plain matmuls/no-ops in between (the PE keeps the
    stationary operand across matmuls; walrus emits one LDWEIGHTS per MATMUL)."""
    n_elided = 0
    for f in nc.m.functions:
        for bb in f.blocks:
            out = []
            last_w = None
            changed = False
            for inst in bb.instructions:
                nm = type(inst).__name__
                if nm == "InstLdweights":
                    si = inst.sync_info
                    clean = not (si and (si.on_wait or si.on_update))
                    w = repr(inst.ins[0])
                    if clean and last_w == w:
                        n_elided += 1
                        changed = True
                        continue
                    last_w = w if clean else None
                elif nm == "InstMatmult":
                    if getattr(inst, "is_transpose", False):
                        last_w = None
                elif nm == "InstNoOp":
                    pass
                else:
                    last_w = None
                out.append(inst)
            if changed:
                bb.instructions = out
    return n_elided


NCORES = 8
N = 16384
SH = N // NCORES  # 2048 adj rows per core
F = 512
HID = 1024
OUT = 256
S2SCALE = 256.0  # s2 is gathered as fp8 of 256*s2

BF16 = mybir.dt.bfloat16
F32 = mybir.dt.float32
FP8 = mybir.dt.float8e4
DR = mybir.MatmulPerfMode.DoubleRow

_built = None


def build():
    nc = bass.Bass()

    # adjU row r = kb*128 + p (k = kb*256 + kk*128 + p global col of adjT_c),
    # col = kk*2048 + i (i = local row of the adj shard), values N*adj in fp8.
    # P1 reads the 1024-wide i-half slices, P2 reads full rows.
    adjU = nc.declare_dram_parameter("adjU", [8192, 2 * SH], FP8, isOutput=False)
    # xP row = kb*128 + p, col = kk*512 + j
    xP = nc.declare_dram_parameter("xP", [8192, 2 * F], FP8, isOutput=False)
    w1n = nc.declare_dram_parameter("w1n", [F, HID], BF16, isOutput=False)  # W1/N
    w2 = nc.declare_dram_parameter("w2", [HID, OUT], BF16, isOutput=False)
    b1T = nc.declare_dram_parameter("b1T", [128, HID // 128], F32, isOutput=False)
    b2T = nc.declare_dram_parameter("b2T", [128, OUT // 128], F32, isOutput=False)
    out2T = nc.declare_dram_parameter("out2T", [OUT, SH], F32, isOutput=True)

    rg = [list(range(NCORES))]

    def allgather(inp, outp):
        return nc.gpsimd.collective_compute(
            "AllGather",
            mybir.AluOpType.bypass,
            replica_groups=rg,
            ins=[inp.opt()],
            outs=[outp.opt()],
        )

    with tile.TileContext(nc) as tc:
        with (
            tc.tile_pool(name="const", bufs=1) as constp,
            tc.tile_pool(name="psum", bufs=8, space="PSUM") as psum,
            tc.tile_pool(name="dram", bufs=1, space="DRAM") as dram,
            tc.tile_pool(name="adj", bufs=6) as adjp,
            tc.tile_pool(name="small", bufs=4) as smallp,
        ):
            # ---- constants / resident tensors ----
            w1t = constp.tile([128, 4, HID], BF16)  # [j%128, jj, hid]
            nc.sync.dma_start(w1t[:], w1n[:].rearrange("(jj p) h -> p jj h", p=128))
            w2t = constp.tile([128, 8, OUT], BF16)  # [hid%128, hh, j2]
            nc.sync.dma_start(w2t[:], w2[:].rearrange("(hh p) o -> p hh o", p=128))
            b1t = constp.tile([128, 8], F32)
            nc.sync.dma_start(b1t[:], b1T[:])
            b2t = constp.tile([128, 2], F32)
            nc.sync.dma_start(b2t[:], b2T[:])
            # all of x resident: [p, kb, kk, j]; loaded in 8 chunks interleaved
            # with the first P1 adj loads so the first matmul starts early
            xt = constp.tile([128, 64, 2, F], FP8)
            xP_r = xP[:].rearrange("(kb p) (kk j) -> p kb kk j", p=128, kk=2)

            # results kept in SBUF
            yT = constp.tile([128, 4, SH], BF16)  # [j%128, jj, i] = N*y
            hT = constp.tile([128, 8, SH], BF16)  # [hid%128, hh, i]

            # AllGather staging: chunk q covers local rows [512q, 512q+512)
            # laid out [p, kbl, kk, j2] (row = kbl*256 + kk*128 + p).
            ag_in = [dram.tile([128, 1024], FP8, name=f"agi{q}") for q in range(4)]
            ag_out = [
                dram.tile([NCORES * 128, 1024], FP8, addr_space="Shared", name=f"ago{q}")
                for q in range(4)
            ]

            adjU_r = adjU[:].rearrange("(kb p) (kk i) -> p kb kk i", p=128, kk=2)

            # P2 is DMA-bandwidth-bound: keep a few adj k-blocks loaded during
            # P1 resident in SBUF so P2 skips their reload (saves 4MB of the
            # ~36MB P2 stream).
            CACHE_KBS = (6, 7, 14, 15, 22, 23, 30, 31)
            adj_cache = {}

            for H in range(2):
                # ---- P1 half H: psY[j, i-1024-half] += x8^T adjC8 ----
                psY = [
                    psum.tile([128, 512], F32, tag="ps", name=f"psY{H}{t}")
                    for t in range(8)
                ]  # tile t = (jj, b): jj*2 + b; holds i-chunks 2b, 2b+1
                for kb in range(64):
                    if H == 0 and kb % 8 == 0:
                        g = kb // 8
                        nc.sync.dma_start(
                            xt[:, g * 8 : g * 8 + 8], xP_r[:, g * 8 : g * 8 + 8]
                        )
                    if kb in CACHE_KBS:
                        at = constp.tile([128, 2, 1024], FP8, name=f"ac_{H}_{kb}")
                        adj_cache[(H, kb)] = at
                    else:
                        at = adjp.tile(
                            [128, 2, 1024], FP8, tag="adjt", name=f"a1_{H}_{kb}"
                        )
                    nc.sync.dma_start(
                        at[:], adjU_r[:, kb, :, H * 1024 : (H + 1) * 1024]
                    )
                    for jj in range(4):
                        lhs = xt[:, kb, :, jj * 128 : (jj + 1) * 128]
                        for c in range(4):  # i-chunk of 256 within the half
                            nc.tensor.matmul(
                                psY[jj * 2 + c // 2][:, (c % 2) * 256 : (c % 2) * 256 + 256],
                                lhs,
                                at[:, :, c * 256 : (c + 1) * 256],
                                start=(kb == 0 and c % 2 == 0),
                                stop=(kb == 63 and c % 2 == 1),
                                perf_mode=DR,
                            )
                # drain psY -> yT (bf16) on the vector engine (ACT is busier)
                for jj in range(4):
                    for b in range(2):
                        nc.vector.tensor_copy(
                            yT[:, jj, H * 1024 + b * 512 : H * 1024 + b * 512 + 512],
                            psY[jj * 2 + b][:],
                        )

                # ---- supportT + relu: hT = relu(W1n^T yT + b1) ----
                # hh-groups of 4 with i-width 1024: each stationary W1 block
                # feeds two 512-wide matmuls, halving LDWEIGHTS count.
                for hg in range(2):
                    i0 = H * 1024
                    psS = [
                        psum.tile([128, 512], F32, tag="ps", name=f"psS{H}{hg}{t}")
                        for t in range(8)
                    ]  # t = hh4*2 + qq
                    for hh4 in range(4):
                        hh = hg * 4 + hh4
                        for jj in range(4):
                            for qq in range(2):
                                nc.tensor.matmul(
                                    psS[hh4 * 2 + qq][:],
                                    w1t[:, jj, hh * 128 : (hh + 1) * 128],
                                    yT[:, jj, i0 + qq * 512 : i0 + qq * 512 + 512],
                                    start=(jj == 0),
                                    stop=(jj == 3),
                                )
                    for hh4 in range(4):
                        hh = hg * 4 + hh4
                        for qq in range(2):
                            nc.scalar.activation(
                                hT[:, hh, i0 + qq * 512 : i0 + qq * 512 + 512],
                                psS[hh4 * 2 + qq][:],
                                mybir.ActivationFunctionType.Relu,
                                bias=b1t[:, hh : hh + 1],
                            )

                # ---- s2 = h @ W2, quantized fp8*256, staged for AG ----
                # psum/AG column order (j2t, kk, jp) so P2's stationary load
                # is a plain contiguous copy.
                for qq in range(2):
                    q = H * 2 + qq  # global chunk id
                    for kbl in range(2):
                        ps2 = psum.tile(
                            [128, 2, 2, 128], F32, tag="ps", name=f"ps2{q}{kbl}"
                        )
                        for kk in range(2):
                            i0 = q * 512 + kbl * 256 + kk * 128
                            for hh in range(8):
                                nc.tensor.matmul(
                                    ps2[:, :, kk, :],
                                    hT[:, hh, i0 : i0 + 128],
                                    w2t[:, hh, :],
                                    start=(hh == 0 and kk == 0),
                                    stop=(hh == 7 and kk == 1),
                                )
                        s2q = smallp.tile([128, 512], FP8, tag="s2q", bufs=2)
                        nc.scalar.activation(
                            s2q[:],
                            ps2[:].rearrange("p a b j -> p (a b j)"),
                            mybir.ActivationFunctionType.Copy,
                            scale=S2SCALE,
                        )
                        nc.scalar.dma_start(
                            ag_in[q][:, kbl * 512 : kbl * 512 + 512], s2q[:]
                        )
                    allgather(ag_in[q], ag_out[q])

            # ---- P2: out2T = (adjU8 @ s2q)^T / (N*256) + b2 ----
            psD = [
                psum.tile([128, 512], F32, tag="ps", name=f"psD{t}") for t in range(8)
            ]  # tile t = (j2, cb): j2*4 + cb; holds i-chunks 2cb, 2cb+1
            kb_order = [
                8 * c + 2 * q + t for q in range(4) for c in range(NCORES) for t in range(2)
            ]
            for ki, kb in enumerate(kb_order):
                c, rem = kb // 8, kb % 8
                q, t = rem // 2, rem % 2
                if kb in CACHE_KBS:
                    at = None  # rhs comes from the SBUF-cached P1 tiles
                else:
                    at = adjp.tile([128, 2, 2048], FP8, tag="adjt", name=f"a2_{kb}")
                    nc.sync.dma_start(at[:], adjU_r[:, kb])
                st = smallp.tile([128, 2, 2, 128], FP8, tag="st", bufs=4, name=f"st{kb}")
                # ag_out rows c*128+p, cols t*512 + j2t*256 + kk*128 + jp
                nc.sync.dma_start(
                    st[:].rearrange("p a b j -> p (a b j)"),
                    ag_out[q][c * 128 : (c + 1) * 128, t * 512 : (t + 1) * 512],
                )
                for j2 in range(2):
                    lhs = st[:, j2]
                    for c8 in range(8):
                        if at is not None:
                            rhs = at[:, :, c8 * 256 : (c8 + 1) * 256]
                        else:
                            cc = c8 % 4
                            rhs = adj_cache[(c8 // 4, kb)][
                                :, :, cc * 256 : (cc + 1) * 256
                            ]
                        nc.tensor.matmul(
                            psD[j2 * 4 + c8 // 2][:, (c8 % 2) * 256 : (c8 % 2) * 256 + 256],
                            lhs,
                            rhs,
                            start=(ki == 0 and c8 % 2 == 0),
                            stop=(ki == 63 and c8 % 2 == 1),
                            perf_mode=DR,
                        )
            # final drain split across ACT and DVE so the tail is ~2x shorter
            for j2 in range(2):
                for cb in range(4):
                    ot = smallp.tile([128, 512], F32, tag="ot", bufs=4)
                    if cb % 2 == 0:
                        nc.scalar.activation(
                            ot[:],
                            psD[j2 * 4 + cb][:],
                            mybir.ActivationFunctionType.Identity,
                            bias=b2t[:, j2 : j2 + 1],
                            scale=1.0 / (N * S2SCALE),
                        )
                        nc.scalar.dma_start(
                            out2T[j2 * 128 : (j2 + 1) * 128, cb * 512 : (cb + 1) * 512],
                            ot[:],
                        )
                    else:
                        nc.vector.tensor_scalar(
                            ot[:],
                            psD[j2 * 4 + cb][:],
                            1.0 / (N * S2SCALE),
                            b2t[:, j2 : j2 + 1],
                            op0=mybir.AluOpType.mult,
                            op1=mybir.AluOpType.add,
                        )
                        nc.gpsimd.dma_start(
                            out2T[j2 * 128 : (j2 + 1) * 128, cb * 512 : (cb + 1) * 512],
                            ot[:],
                        )

    _elide_redundant_ldweights(nc)
    _split_excess_waits(nc)
    return nc


def _prep_inputs(x, adj, W1, b1, W2, b2):
    bf = ml_dtypes.bfloat16
    f8 = ml_dtypes.float8_e4m3fn

    u = adj * np.float32(N)  # exact: adj was u/N with N a power of two
    u8 = u.astype(f8)
    x8 = x.astype(f8)
    b1T = np.ascontiguousarray(b1.reshape(HID // 128, 128).T).astype(np.float32)
    b2T = np.ascontiguousarray(b2.reshape(OUT // 128, 128).T).astype(np.float32)
    w1n = (W1 / np.float32(N)).astype(bf)
    w2b = W2.astype(bf)
    # xP[kb*128+p, kk*512+j] = x8[kb*256+kk*128+p, j]
    xP = np.ascontiguousarray(
        x8.reshape(64, 2, 128, F).transpose(0, 2, 1, 3).reshape(8192, 2 * F)
    )

    def adj_layout(a8, rows):
        # out[kb*128+p, kk*2048+i] = a8[rows][i, kb*256+kk*128+p]
        blk = a8[rows, :].reshape(SH, 64, 2, 128)  # [i, kb, kk, p]
        return np.ascontiguousarray(
            blk.transpose(1, 3, 2, 0).reshape(8192, 2 * SH)
        )

    in_maps = []
    for c in range(NCORES):
        rows = slice(c * SH, (c + 1) * SH)
        in_maps.append(
            {
                "adjU": adj_layout(u8, rows),
                "xP": xP,
                "w1n": w1n,
                "w2": w2b,
                "b1T": b1T,
                "b2T": b2T,
            }
        )
    return in_maps


def _run(inputs, trace=False):
    global _built
    if _built is None:
        _built = build()
    in_maps = _prep_inputs(**inputs)
    r = run_bass_kernel_spmd(_built, in_maps, list(range(NCORES)), trace=trace)
    out = np.empty([N, OUT], np.float32)
    for c in range(NCORES):
        out[c * SH : (c + 1) * SH, :] = r.results[c]["out2T"].T
    return out, r


def kernel(x, adj, W1, b1, W2, b2):
    out, _ = _run(dict(x=x, adj=adj, W1=W1, b1=b1, W2=W2, b2=b2))
    return out
